# revision 1
# baseline (speedup 1.0000x reference)
"""Self-contained Trainium2 Bass kernel for the 2-layer GAT problem.

Accepts FULL inputs (as produced by setup_inputs()), shards across the
8 NeuronCores internally, returns the full [100000, 1] float32 output.

Structure: host index-preprocessing (dst-shard / src-chunk streams) +
Bass/Tile SPMD kernel (ap_gather streams, segmented scans, PE collapse,
AllGather between layers). Exploits x being [N,1]: layer-1 messages are
rank-1 in x; with b1 == 0 layer-2 messages are rank-2 in relu(+-P).
Falls back to a numpy reference path if b1/b2 are nonzero (not the case
for this problem's inputs).
"""
import numpy as np
import ml_dtypes

N_NODES = 100000

# ===================== tile drain workaround =====================
"""Tile drain workaround: this walrus build allows at most 1 sync-wait on SP CTRL
instructions, but TileContext's tail drain aggregates all end-of-kernel
waits onto one Drain. Split them across nops instead."""
import concourse.tile as tile
from concourse import mybir
from bass_rust import ScopedClock

def _patched_drain_and_barrier(self, tick_clock, wait_clock):
    nc = self.nc
    probe = nc.sync.nop()
    wait_clock.add_sem_waits(probe.ins, ScopedClock({None: tick_clock.global_clock}))
    si = probe.ins.sync_info
    waits = list(si.on_wait) if si is not None else []
    if si is not None:
        si.on_wait = waits[:1]
    for w in waits[1:]:
        nop = nc.sync.nop()
        nop.ins.sync_info = mybir.SyncInfo(on_wait=[w], on_update=[])
    nc.sync.drain()
    nc.all_engine_barrier()
    popped = nc._tile_sem_poison_stack.pop()
    assert popped is self._sem_poison
    nc.clear_and_free_semaphores(list(self.sems.allocated().values()))
    nc.all_engine_barrier()

def install():
    tile.TileContext._drain_and_barrier = _patched_drain_and_barrier


# ===================== host preprocessing =====================
import numpy as np


SENT = -1  # placeholder; real sentinel index = Nsh (last table row, value 0)


def prep(edge_index: np.ndarray, N: int, n_cores: int = 8, n_groups: int = 8,
         sub_pad: int = 8):
    """Build all index structures. edge_index: [2, E] ints (no self loops yet).

    Returns dict with per-core arrays (lists over cores).
    """
    E = edge_index.shape[1]
    assert N % n_cores == 0 and n_cores == 8 and n_groups == 8
    Nsh = N // n_cores

    loops = np.arange(N, dtype=np.int64)
    src = np.concatenate([edge_index[0], loops]).astype(np.int64)
    dst = np.concatenate([edge_index[1], loops]).astype(np.int64)

    core = dst // Nsh          # dst shard
    grp = src // Nsh           # src chunk
    # sort all edges by (core, group, dst, src) with one argsort
    key = ((core * 8 + grp) * N + dst)
    order = np.argsort(key, kind="stable")
    src_s, dst_s = src[order], dst[order]
    core_s, grp_s = core[order], grp[order]

    # per (core, group) segment boundaries in the sorted edge array
    cg = core_s * 8 + grp_s
    cg_starts = np.searchsorted(cg, np.arange(64))
    cg_ends = np.searchsorted(cg, np.arange(64), side="right")

    # ---- determine L16 (same for all (c,g)) ----
    # slots per (c,g) + per-subchunk padding waste; choose L16 with slack, then
    # verify greedy packing fits; grow if needed.
    max_slots = int((cg_ends - cg_starts).max())
    L16 = -(-max_slots // 16) + sub_pad  # slack for run-boundary padding
    L16 = -(-L16 // 16) * 16  # multiple of 16 (wrapped idx layout)

    Nshp = -(-Nsh // 128) * 128  # padded dst count (dummy dsts at end)
    W = Nshp // 128              # per-partition width of per-dst layout

    cores_out = []
    for c in range(n_cores):
        while True:
            ok, out = _pack_core(src_s, dst_s, cg_starts, cg_ends, c, Nsh,
                                 Nshp, L16)
            if ok:
                break
            L16 += 16
        cores_out.append(out)
        # note: if a later core needs bigger L16, repack earlier ones
    # ensure uniform L16 (repack all with final L16)
    final_L16 = L16
    cores_out = []
    for c in range(n_cores):
        ok, out = _pack_core(src_s, dst_s, cg_starts, cg_ends, c, Nsh, Nshp,
                             final_L16)
        assert ok
        cores_out.append(out)

    return dict(N=N, E2=E + N, Nsh=Nsh, Nshp=Nshp, W=W, L16=final_L16,
                cores=cores_out)


def _pack_core(src_s, dst_s, cg_starts, cg_ends, c, Nsh, Nshp, L16):
    """Pack one core's 8 group-streams. Returns (ok, dict)."""
    n_groups = 8
    src_idx = np.full((n_groups, 16, L16), Nsh, dtype=np.int16)   # sentinel
    dst_idx = np.full((n_groups, 16, L16), Nsh, dtype=np.int16)
    runstart = np.ones((n_groups, 16, L16), dtype=np.float32)
    ends_f = np.zeros((n_groups, Nshp), dtype=np.int16)
    M = np.zeros((128, Nshp), dtype=np.float32)

    for g in range(n_groups):
        s0, s1 = cg_starts[c * 8 + g], cg_ends[c * 8 + g]
        sseg = src_s[s0:s1] - g * Nsh      # src_local
        dseg = dst_s[s0:s1] - c * Nsh      # dst_local
        n = s1 - s0
        if n == 0:
            continue
        # run boundaries (dseg sorted)
        newrun = np.empty(n, dtype=bool)
        newrun[0] = True
        np.not_equal(dseg[1:], dseg[:-1], out=newrun[1:])
        run_starts = np.flatnonzero(newrun)          # index into seg
        run_dst = dseg[run_starts]                   # dst_local per run
        run_lens = np.diff(np.append(run_starts, n))
        nruns = len(run_starts)

        # greedy assignment of runs to 16 subchunks of capacity L16
        # vectorized: cumulative position with bumps at capacity crossings
        pos = np.zeros(nruns, dtype=np.int64)  # start slot (global over 16*L16)
        cum = 0
        # iterate subchunks, placing a prefix of remaining runs in each
        r = 0
        ends_pos = np.empty(nruns, dtype=np.int64)
        for sc in range(16):
            cap_end = (sc + 1) * L16
            base = sc * L16
            cum = base
            # place runs while they fit
            rl = run_lens[r:]
            if len(rl) == 0:
                break
            fit = np.cumsum(rl)
            k = int(np.searchsorted(fit, L16, side="right"))
            if k == 0:
                return False, None  # single run longer than L16
            take = slice(r, r + k)
            offs = np.concatenate([[0], fit[:k - 1]])
            pos[take] = base + offs
            ends_pos[take] = base + fit[:k] - 1
            r += k
        if r < nruns:
            return False, None  # didn't fit

        # fill slot arrays
        slot_of_edge = np.repeat(pos, run_lens) + _intra_run_offsets(run_lens)
        p_of_slot = slot_of_edge // L16
        f_of_slot = slot_of_edge % L16
        src_idx[g, p_of_slot, f_of_slot] = sseg.astype(np.int16)
        dst_idx[g, p_of_slot, f_of_slot] = dseg.astype(np.int16)
        runstart[g, pos // L16, pos % L16] = 0.0

        # ends per dst
        ends_f[g, run_dst] = (ends_pos % L16).astype(np.int16)
        M[16 * g + (ends_pos // L16), run_dst] = 1.0

    # wrapped int16 idx layout for ap_gather: idx i -> partition i%16, col i//16
    # per-subchunk gather: num_idxs = L16 per group; stream for (g, sc) is
    # src_idx[g, sc, :].  We emit [128, L16//16] per subchunk instruction:
    # partition 16g+p holds entries i with i%16==p of group g's subchunk-sc
    # stream.
    def wrap_subchunk(a):  # a: [8, 16, L16] -> [16(sc), 128, L16//16]
        out = np.zeros((16, 128, -(-L16 // 16)), dtype=np.int16)
        for sc in range(16):
            for g in range(8):
                stream = a[g, sc]
                L = len(stream)
                padded = np.zeros((-(-L // 16) * 16,), dtype=np.int16)
                padded[:L] = stream
                out[sc, 16 * g:16 * g + 16, :] = padded.reshape(-1, 16).T
        return out

    Nw = Nshp // 16

    def wrap_ends(a):  # [8, Nshp] -> [128, Nshp//16]
        out = np.zeros((128, Nw), dtype=np.int16)
        for g in range(8):
            out[16 * g:16 * g + 16, :] = a[g].reshape(-1, 16).T
        return out

    return True, dict(
        src_idx=src_idx, dst_idx=dst_idx, runstart=runstart,
        src_idx_w=wrap_subchunk(src_idx), dst_idx_w=wrap_subchunk(dst_idx),
        ends_w=wrap_ends(ends_f), M=M,
    )


def _intra_run_offsets(run_lens):
    # offsets 0..len-1 within each run, vectorized
    total = int(run_lens.sum())
    idx = np.arange(total, dtype=np.int64)
    starts = np.repeat(np.cumsum(np.append(0, run_lens[:-1])), run_lens)
    return idx - starts




# ===================== bass kernel builder =====================
from contextlib import ExitStack
import concourse.bass as bass
import concourse.bacc as bacc
import concourse.tile as tile
from concourse import mybir
from concourse import library_config
install()


F32 = mybir.dt.float32
I16 = mybir.dt.int16
BF16 = mybir.dt.bfloat16
AF = mybir.ActivationFunctionType
ALU = mybir.AluOpType
AX = mybir.AxisListType


def build(pp, dbg=False, runstart_f32=False, no_collective=False):
    N, Nsh, Nshp, W, L16 = pp["N"], pp["Nsh"], pp["Nshp"], pp["W"], pp["L16"]
    SH = Nsh + (-Nsh) % 16          # shard upload length
    NT = Nsh + 16                   # gather table width (sentinel at Nsh)
    NF = -(-N // 128)               # x_full cols as [128, NF]
    PF = (8 * SH) // 128            # p_full cols as [128, PF]
    LW = L16 // 16
    EW = Nshp // 16
    assert L16 % 16 == 0 and Nshp % 128 == 0 and (8 * SH) % 128 == 0
    assert NT <= 32768 and L16 <= 32768

    BW = 448 if Nshp % 448 == 0 else Nshp // 4   # boundary dst-chunk width
    while Nshp % BW or BW % 16:
        BW //= 2
    NBC = Nshp // BW

    nc = bacc.Bacc("TRN2", target_bir_lowering=False, debug=False,
                   num_devices=8)

    def din(name, shape, dt=F32):
        return nc.dram_tensor(name, shape, dt, kind="ExternalInput")

    x_full = din("x_full", [1, 128 * NF])
    x_shard = din("x_shard", [1, SH])
    W1 = din("W1", [1, 20])
    a_src1 = din("a_src1", [1, 20])
    a_dst1 = din("a_dst1", [1, 20])
    W2T = din("W2T", [1, 400])      # W2.T row-major [j, k]
    a_src2 = din("a_src2", [1, 20])
    a_dst2 = din("a_dst2", [1, 20])
    b2 = din("b2", [1, 20])
    Wl = din("Wl", [1, 20])
    bl = din("bl", [1, 1])
    srcw = din("src_idx_w", [16 * 128, LW], I16)
    dstw = din("dst_idx_w", [16 * 128, LW], I16)
    endsw = din("ends_w", [128, EW], I16)
    Mmask = din("M", [128, Nshp])
    runst = din("runstart", [128, L16], F32 if runstart_f32 else BF16)

    y_out = nc.dram_tensor("y", [1, Nshp], F32, kind="ExternalOutput")
    dbg_names = ["den1", "P1", "Pn", "den2", "Rp", "Rm"] if dbg else []
    dbg_outs = {nm: nc.dram_tensor("dbg_" + nm, [1, Nshp], F32,
                                   kind="ExternalOutput")
                for nm in dbg_names}

    p_local = nc.dram_tensor("p_local", [1, SH], F32)
    p_full = nc.dram_tensor("p_full", [1, 8 * SH], F32, addr_space="Shared")
    v2_local = nc.dram_tensor("v2_local", [1, SH], F32)
    rt128 = nc.dram_tensor("rt128", [1, 128], F32)
    rt1 = nc.dram_tensor("rt1", [1, 1], F32)
    sums_d = [nc.dram_tensor(f"sums{i}", [1, Nshp], F32) for i in range(5)]

    with tile.TileContext(nc) as tc, ExitStack() as ctx:
        consts = ctx.enter_context(tc.tile_pool(name="consts", bufs=1))
        smalls = ctx.enter_context(tc.tile_pool(name="smalls", bufs=2))
        tabp = ctx.enter_context(tc.tile_pool(name="tables", bufs=1))
        strm = ctx.enter_context(tc.tile_pool(name="strm", bufs=5))
        idxp = ctx.enter_context(tc.tile_pool(name="idx", bufs=2))
        bnd = ctx.enter_context(tc.tile_pool(name="bnd", bufs=2))
        nodep = ctx.enter_context(tc.tile_pool(name="node", bufs=1))
        psp = ctx.enter_context(tc.tile_pool(name="ps", bufs=2, space="PSUM"))

        def S(name):
            return strm.tile([128, L16], F32, tag="s", name=name, bufs=5)

        def bcast(dram_ap, n, name):
            t = consts.tile([128, n], F32, name=name)
            nc.sync.dma_start(t[:], dram_ap.partition_broadcast(128))
            return t

        def rsc(t, name, op=ALU.add):
            out = consts.tile([128, 1], F32, name=name)
            nc.vector.tensor_reduce(out[:], t[:], AX.X, op)
            return out

        def tmul(a, b, name, pool=None):
            out = (pool or smalls).tile(list(a.shape), F32, name=name)
            nc.vector.tensor_tensor(out=out[:], in0=a[:], in1=b[:],
                                    op=ALU.mult)
            return out

        # ------------- constants -------------
        wb = bcast(W1.ap(), 20, "wb")
        a1s = bcast(a_src1.ap(), 20, "a1s")
        a1d = bcast(a_dst1.ap(), 20, "a1d")
        a2s = bcast(a_src2.ap(), 20, "a2s")
        a2d = bcast(a_dst2.ap(), 20, "a2d")
        b2t = bcast(b2.ap(), 20, "b2t")
        wlt = bcast(Wl.ap(), 20, "wlt")
        blt = bcast(bl.ap(), 1, "blt")
        w2t = bcast(W2T.ap(), 400, "w2t")

        c1 = rsc(tmul(wb, a1s, "c1m"), "c1")
        c2 = rsc(tmul(wb, a1d, "c2m"), "c2")

        wp = consts.tile([128, 20], F32, name="wp")
        nc.scalar.activation(wp[:], wb[:], AF.Relu)
        wm = consts.tile([128, 20], F32, name="wm")
        nc.scalar.activation(wm[:], wb[:], AF.Relu, scale=-1.0)

        def qvec(wv, name):
            t = smalls.tile([128, 400], F32, name=name + "_t", tag="q400")
            nc.vector.tensor_tensor(
                out=t[:], in0=w2t[:],
                in1=wv[:].unsqueeze(1).broadcast_to([128, 20, 20]),
                op=ALU.mult)
            out = consts.tile([128, 20], F32, name=name)
            nc.vector.tensor_reduce(
                out[:], t[:].rearrange("p (j k) -> p j k", j=20), AX.X,
                ALU.add)
            return out

        qp = qvec(wp, "qp")
        qm = qvec(wm, "qm")
        A2 = rsc(tmul(qp, a2s, "A2m"), "A2")
        B2 = rsc(tmul(qm, a2s, "B2m"), "B2")
        C2 = rsc(tmul(qp, a2d, "C2m"), "C2")
        D2 = rsc(tmul(qm, a2d, "D2m"), "D2")

        ones128 = consts.tile([128, 1], F32, name="ones128")
        nc.vector.memset(ones128[:], 1.0)

        def cross_max(tin, name):
            """[128, n] -> global max over everything, bcast [128, 1]."""
            m = smalls.tile([128, 1], F32, name=name + "_m")
            nc.vector.tensor_reduce(m[:], tin[:], AX.X, ALU.max)
            nc.sync.dma_start(rt128.ap(), m[:])
            row = smalls.tile([1, 128], F32, name=name + "_row")
            nc.sync.dma_start(row[:], rt128.ap())
            m1 = smalls.tile([1, 1], F32, name=name + "_m1")
            nc.vector.tensor_reduce(m1[:], row[:], AX.X, ALU.max)
            nc.sync.dma_start(rt1.ap(), m1[:])
            mb = consts.tile([128, 1], F32, name=name)
            nc.sync.dma_start(mb[:], rt1.ap().partition_broadcast(128))
            return mb

        def relu_pair_max(dram_ap, ncols, name):
            big = nodep.tile([128, ncols], F32, tag="gmax", name=name + "_big",
                             bufs=2)
            nc.sync.dma_start(big[:], dram_ap)
            outs = []
            for i, sgn in enumerate((1.0, -1.0)):
                r = nodep.tile([128, ncols], F32, tag="gmax",
                               name=f"{name}_r{i}", bufs=2)
                nc.scalar.activation(r[:], big[:], AF.Relu, scale=sgn)
                outs.append(cross_max(r, f"{name}{i}"))
            return outs

        def sc1(name):
            return consts.tile([128, 1], F32, name=name)

        def lrelu_neg(t, name):
            o = sc1(name + "_lr")
            nc.vector.scalar_tensor_tensor(out=o[:], in0=t[:], scalar=0.2,
                                           in1=t[:], op0=ALU.mult,
                                           op1=ALU.max)
            o2 = sc1(name)
            nc.scalar.mul(o2[:], o[:], -1.0)
            return o2

        mxp, mxm = relu_pair_max(x_full.ap(), NF, "mx")

        def ub_exact(cc, name):
            # max(c*mp, -c*mm)
            t1 = tmul(cc, mxp, name + "_1")
            ncc = smalls.tile([128, 1], F32, name=name + "_n")
            nc.scalar.mul(ncc[:], cc[:], -1.0)
            t2 = tmul(ncc, mxm, name + "_2")
            o = smalls.tile([128, 1], F32, name=name)
            nc.vector.tensor_tensor(out=o[:], in0=t1[:], in1=t2[:], op=ALU.max)
            return o

        ub1 = smalls.tile([128, 1], F32, name="ub1")
        nc.vector.tensor_tensor(out=ub1[:], in0=ub_exact(c1, "ubu1")[:],
                                in1=ub_exact(c2, "ubv1")[:], op=ALU.add)
        gneg1 = lrelu_neg(ub1, "gneg1")

        # ------------- shared edge-phase pieces -------------
        runstart_t = nodep.tile([128, L16], F32 if runstart_f32 else BF16,
                                name="runstart_t")
        nc.sync.dma_start(runstart_t[:], runst.ap())
        ends_t = nodep.tile([128, EW], I16, name="ends_t")
        nc.sync.dma_start(ends_t[:], endsw.ap())

        def gather_pass(table, idx_dram, name):
            packed = strm.tile([128, L16], F32, tag="pk", name=name, bufs=2)
            for sc in range(16):
                it = idxp.tile([128, LW], I16, tag="it", name=f"{name}_i{sc}")
                nc.sync.dma_start(it[:], idx_dram[128 * sc:128 * (sc + 1), :])
                gt = strm.tile([128, L16], F32, tag="s", bufs=5,
                               name=f"{name}_g{sc}")
                nc.gpsimd.ap_gather(
                    gt[:], table[:], it[:],
                    channels=128, num_elems=NT, d=1, num_idxs=L16)
                for g in range(8):
                    eng = nc.sync if g % 2 == 0 else nc.scalar
                    eng.dma_start(packed[16 * g + sc:16 * g + sc + 1, :],
                                  gt[16 * g:16 * g + 1, :])
            return packed

        def seg_scan(data, name):
            s = S(name)
            nc.vector.tensor_tensor_scan(
                s[:], runstart_t[:], data[:], 0.0, ALU.mult, ALU.add)
            return s

        def boundary(chans, sums_slots):
            for bc in range(NBC):
                me = bnd.tile([128, BW], F32, tag="me", name=f"me{bc}")
                nc.sync.dma_start(me[:], Mmask.ap()[:, bc * BW:(bc + 1) * BW])
                for ci, s in enumerate(chans):
                    cd = bnd.tile([128, BW], F32, tag="cd",
                                  name=f"cd{bc}_{ci}")
                    nc.gpsimd.ap_gather(
                        cd[:], s[:],
                        ends_t[:, bc * (BW // 16):(bc + 1) * (BW // 16)],
                        channels=128, num_elems=L16, d=1, num_idxs=BW)
                    mk = bnd.tile([128, BW], F32, tag="mk",
                                  name=f"mk{bc}_{ci}")
                    nc.vector.tensor_tensor(out=mk[:], in0=cd[:], in1=me[:],
                                            op=ALU.mult)
                    for k0 in range(0, BW, 512):
                        kn = min(512, BW - k0)
                        ps = psp.tile([1, kn], F32, tag="ps",
                                      name=f"ps{bc}_{ci}_{k0}")
                        nc.tensor.matmul(ps[:], ones128[:],
                                         mk[:, k0:k0 + kn], start=True,
                                         stop=True)
                        ev = bnd.tile([1, kn], F32, tag="ev",
                                      name=f"ev{bc}_{ci}_{k0}")
                        nc.scalar.copy(ev[:], ps[:])
                        nc.sync.dma_start(
                            sums_d[sums_slots[ci]].ap()[:, bc * BW + k0:
                                                        bc * BW + k0 + kn],
                            ev[:])

        def sums_to_128(slot, name):
            o = smalls.tile([128, W], F32, name=name, tag="nw", bufs=6)
            nc.sync.dma_start(o[:], sums_d[slot].ap())
            return o

        def dump_dbg(nm, t):
            if nm in dbg_outs:
                nc.sync.dma_start(dbg_outs[nm].ap(), t[:])

        # ------------- layer 1 -------------
        tsx = tabp.tile([128, NT], F32, tag="table", name="tsx")
        for g in range(8):
            nc.sync.dma_start(
                tsx[16 * g:16 * (g + 1), 0:Nsh],
                x_full.ap()[:, g * Nsh:(g + 1) * Nsh].partition_broadcast(16))
        nc.vector.memset(tsx[:, Nsh:NT], 0.0)
        psrc = gather_pass(tsx, srcw.ap(), "psrc")

        tvx = tabp.tile([128, NT], F32, tag="table", name="tvx")
        nc.sync.dma_start(tvx[:, 0:Nsh],
                          x_shard.ap()[:, 0:Nsh].partition_broadcast(128))
        nc.vector.memset(tvx[:, Nsh:NT], 0.0)
        pv = gather_pass(tvx, dstw.ap(), "pv")

        tmp = S("tmp1")
        nc.vector.tensor_scalar(out=tmp[:], in0=pv[:], scalar1=c2[:],
                                scalar2=None, op0=ALU.mult)
        epre = S("epre")
        nc.vector.scalar_tensor_tensor(out=epre[:], in0=psrc[:], scalar=c1[:],
                                       in1=tmp[:], op0=ALU.mult, op1=ALU.add)
        ae = S("ae")
        nc.vector.scalar_tensor_tensor(out=ae[:], in0=epre[:], scalar=0.2,
                                       in1=epre[:], op0=ALU.mult, op1=ALU.max)
        numer = S("numer")
        nc.scalar.activation(numer[:], ae[:], AF.Exp, bias=gneg1[:])
        w1 = S("w1")
        nc.vector.tensor_tensor(out=w1[:], in0=numer[:], in1=psrc[:],
                                op=ALU.mult)
        s0 = seg_scan(numer, "s0")
        s1 = seg_scan(w1, "s1")
        boundary([s0, s1], [0, 1])

        den1 = sums_to_128(0, "den1")
        P1 = sums_to_128(1, "P1")
        den1e = smalls.tile([128, W], F32, name="den1e", tag="nw", bufs=6)
        nc.vector.tensor_scalar(out=den1e[:], in0=den1[:], scalar1=1e-30,
                                scalar2=None, op0=ALU.add)
        rec1 = smalls.tile([128, W], F32, name="rec1", tag="nw", bufs=6)
        nc.vector.reciprocal(rec1[:], den1e[:])
        Pn = nodep.tile([128, W], F32, name="Pn")
        nc.vector.tensor_tensor(out=Pn[:], in0=P1[:], in1=rec1[:],
                                op=ALU.mult)
        # zero dummy-dst tail (partition 127, cols beyond real shard)
        if Nsh < 128 * W:
            zt = smalls.tile([1, W], F32, name="zt")
            nc.vector.memset(zt[:], 0.0)
            for p in range(Nsh // W, 128):
                a = max(0, Nsh - p * W)
                if a < W:
                    nc.sync.dma_start(Pn[p:p + 1, a:W], zt[0:1, a:W])

        dump_dbg("den1", den1)
        dump_dbg("P1", P1)
        dump_dbg("Pn", Pn)

        fp = SH // W            # partitions fully covered by p_local
        rem = SH - fp * W
        nc.sync.dma_start(p_local.ap()[:, 0:fp * W], Pn[0:fp, :])
        if rem:
            nc.sync.dma_start(p_local.ap()[:, fp * W:SH],
                              Pn[fp:fp + 1, 0:rem])

        if no_collective:
            for cc_ in range(8):
                nc.sync.dma_start(p_full.ap()[:, cc_ * SH:(cc_ + 1) * SH],
                                  p_local.ap())
        else:
            nc.gpsimd.collective_compute(
                "AllGather", ALU.bypass, replica_groups=[list(range(8))],
                ins=[p_local.ap()], outs=[p_full.ap()])

        # ------------- layer 2 node arrays -------------
        rpn = smalls.tile([128, W], F32, name="rpn", tag="nw", bufs=6)
        nc.scalar.activation(rpn[:], Pn[:], AF.Relu)
        rmn = smalls.tile([128, W], F32, name="rmn", tag="nw", bufs=6)
        nc.scalar.activation(rmn[:], Pn[:], AF.Relu, scale=-1.0)
        v2a = smalls.tile([128, W], F32, name="v2a", tag="nw", bufs=6)
        nc.vector.tensor_scalar(out=v2a[:], in0=rpn[:], scalar1=C2[:],
                                scalar2=None, op0=ALU.mult)
        v2sh = smalls.tile([128, W], F32, name="v2sh", tag="nw", bufs=6)
        nc.vector.scalar_tensor_tensor(out=v2sh[:], in0=rmn[:], scalar=D2[:],
                                       in1=v2a[:], op0=ALU.mult, op1=ALU.add)
        nc.sync.dma_start(v2_local.ap()[:, 0:fp * W], v2sh[0:fp, :])
        if rem:
            nc.sync.dma_start(v2_local.ap()[:, fp * W:SH],
                              v2sh[fp:fp + 1, 0:rem])

        big2 = nodep.tile([128, PF], F32, tag="gmax", name="pf_big", bufs=2)
        nc.sync.dma_start(big2[:], p_full.ap())
        mpp_i = nodep.tile([128, PF], F32, tag="gmax", name="pf_rp", bufs=2)
        nc.scalar.activation(mpp_i[:], big2[:], AF.Relu)
        mpm_i = nodep.tile([128, PF], F32, tag="gmax", name="pf_rm", bufs=2)
        nc.scalar.activation(mpm_i[:], big2[:], AF.Relu, scale=-1.0)
        mpp = cross_max(mpp_i, "mpp")
        mpm = cross_max(mpm_i, "mpm")

        def ub_pos(ca, cb, name):
            t1 = tmul(ca, mpp, name + "_1")
            r1 = smalls.tile([128, 1], F32, name=name + "_r1")
            nc.scalar.activation(r1[:], t1[:], AF.Relu)
            t2 = tmul(cb, mpm, name + "_2")
            r2 = smalls.tile([128, 1], F32, name=name + "_r2")
            nc.scalar.activation(r2[:], t2[:], AF.Relu)
            o = smalls.tile([128, 1], F32, name=name)
            nc.vector.tensor_tensor(out=o[:], in0=r1[:], in1=r2[:], op=ALU.add)
            return o

        ub2 = smalls.tile([128, 1], F32, name="ub2")
        nc.vector.tensor_tensor(out=ub2[:], in0=ub_pos(A2, B2, "ubu2")[:],
                                in1=ub_pos(C2, D2, "ubv2")[:], op=ALU.add)
        gneg2 = lrelu_neg(ub2, "gneg2")

        # ------------- layer 2 tables + edges -------------
        tP = tabp.tile([128, NT], F32, tag="table", name="tP")
        for g in range(8):
            nc.sync.dma_start(
                tP[16 * g:16 * (g + 1), 0:Nsh],
                p_full.ap()[:, g * SH:g * SH + Nsh].partition_broadcast(16))
        nc.vector.memset(tP[:, Nsh:NT], 0.0)
        psrc2 = gather_pass(tP, srcw.ap(), "psrc2")

        tv2 = tabp.tile([128, NT], F32, tag="table", name="tv2")
        nc.sync.dma_start(tv2[:, 0:Nsh],
                          v2_local.ap()[:, 0:Nsh].partition_broadcast(128))
        nc.vector.memset(tv2[:, Nsh:NT], 0.0)
        pv2 = gather_pass(tv2, dstw.ap(), "pv2")

        rp = S("rp")
        nc.scalar.activation(rp[:], psrc2[:], AF.Relu)
        rm = S("rm")
        nc.scalar.activation(rm[:], psrc2[:], AF.Relu, scale=-1.0)
        u2t = S("u2t")
        nc.vector.tensor_scalar(out=u2t[:], in0=rp[:], scalar1=A2[:],
                                scalar2=None, op0=ALU.mult)
        u2 = S("u2")
        nc.vector.scalar_tensor_tensor(out=u2[:], in0=rm[:], scalar=B2[:],
                                       in1=u2t[:], op0=ALU.mult, op1=ALU.add)
        epre2 = S("epre2")
        nc.vector.tensor_tensor(out=epre2[:], in0=u2[:], in1=pv2[:],
                                op=ALU.add)
        ae2 = S("ae2")
        nc.vector.scalar_tensor_tensor(out=ae2[:], in0=epre2[:], scalar=0.2,
                                       in1=epre2[:], op0=ALU.mult,
                                       op1=ALU.max)
        numer2 = S("numer2")
        nc.scalar.activation(numer2[:], ae2[:], AF.Exp, bias=gneg2[:])
        w21 = S("w21")
        nc.vector.tensor_tensor(out=w21[:], in0=numer2[:], in1=rp[:],
                                op=ALU.mult)
        w22 = S("w22")
        nc.vector.tensor_tensor(out=w22[:], in0=numer2[:], in1=rm[:],
                                op=ALU.mult)
        t0 = seg_scan(numer2, "t0")
        t1 = seg_scan(w21, "t1")
        t2 = seg_scan(w22, "t2")
        boundary([t0, t1, t2], [2, 3, 4])

        den2 = sums_to_128(2, "den2")
        Sp = sums_to_128(3, "Sp")
        Sm = sums_to_128(4, "Sm")
        den2e = smalls.tile([128, W], F32, name="den2e", tag="nw", bufs=6)
        nc.vector.tensor_scalar(out=den2e[:], in0=den2[:], scalar1=1e-30,
                                scalar2=None, op0=ALU.add)
        rec2 = smalls.tile([128, W], F32, name="rec2", tag="nw", bufs=6)
        nc.vector.reciprocal(rec2[:], den2e[:])
        Rp = smalls.tile([128, W], F32, name="Rp", tag="nw", bufs=6)
        nc.vector.tensor_tensor(out=Rp[:], in0=Sp[:], in1=rec2[:],
                                op=ALU.mult)
        Rm = smalls.tile([128, W], F32, name="Rm", tag="nw", bufs=6)
        nc.vector.tensor_tensor(out=Rm[:], in0=Sm[:], in1=rec2[:],
                                op=ALU.mult)

        dump_dbg("den2", den2)
        dump_dbg("Rp", Rp)
        dump_dbg("Rm", Rm)

        # y[d] = bl + sum_k relu(Rp*qp_k + Rm*qm_k + b2_k) * Wl_k
        yacc = smalls.tile([128, W], F32, name="yacc", tag="nw", bufs=6)
        nc.vector.memset(yacc[:], 0.0)
        for k in range(20):
            tk = smalls.tile([128, W], F32, name=f"yk{k}", tag="yk", bufs=3)
            nc.vector.tensor_scalar(out=tk[:], in0=Rp[:],
                                    scalar1=qp[:, k:k + 1], scalar2=None,
                                    op0=ALU.mult)
            nc.vector.scalar_tensor_tensor(out=tk[:], in0=Rm[:],
                                           scalar=qm[:, k:k + 1], in1=tk[:],
                                           op0=ALU.mult, op1=ALU.add)
            hk = smalls.tile([128, W], F32, name=f"hk{k}", tag="yk", bufs=3)
            nc.scalar.activation(hk[:], tk[:], AF.Relu,
                                 bias=b2t[:, k:k + 1])
            nc.vector.scalar_tensor_tensor(out=yacc[:], in0=hk[:],
                                           scalar=wlt[:, k:k + 1],
                                           in1=yacc[:], op0=ALU.mult,
                                           op1=ALU.add)
        yf = smalls.tile([128, W], F32, name="yf", tag="nw", bufs=6)
        nc.vector.tensor_scalar(out=yf[:], in0=yacc[:], scalar1=blt[:],
                                scalar2=None, op0=ALU.add)
        nc.sync.dma_start(y_out.ap(), yf[:])

    nc.compile()
    return nc


def make_in_maps(pp, inputs):
    N, Nsh, Nshp, L16 = pp["N"], pp["Nsh"], pp["Nshp"], pp["L16"]
    SH = Nsh + (-Nsh) % 16
    NF = -(-N // 128)
    x = np.asarray(inputs["x"], np.float32).reshape(-1)
    x_full = np.zeros(128 * NF, np.float32)
    x_full[:N] = x
    W2T = np.ascontiguousarray(np.asarray(inputs["W2"], np.float32).T)

    common = {
        "x_full": x_full[None, :],
        "W1": np.asarray(inputs["W1"], np.float32).reshape(1, 20),
        "a_src1": np.asarray(inputs["a_src1"], np.float32).reshape(1, 20),
        "a_dst1": np.asarray(inputs["a_dst1"], np.float32).reshape(1, 20),
        "W2T": W2T.reshape(1, 400),
        "a_src2": np.asarray(inputs["a_src2"], np.float32).reshape(1, 20),
        "a_dst2": np.asarray(inputs["a_dst2"], np.float32).reshape(1, 20),
        "b2": np.asarray(inputs["b2"], np.float32).reshape(1, 20),
        "Wl": np.asarray(inputs["Wl"], np.float32).reshape(1, 20),
        "bl": np.asarray(inputs["bl"], np.float32).reshape(1, 1),
    }
    maps = []
    for c in range(8):
        pc = pp["cores"][c]
        xs = np.zeros(SH, np.float32)
        xs[:Nsh] = x[c * Nsh:(c + 1) * Nsh]
        LW = L16 // 16
        rs = np.zeros((128, L16), np.float32)
        for g in range(8):
            rs[16 * g:16 * (g + 1), :] = pc["runstart"][g]
        maps.append({
            **common,
            "x_shard": xs[None, :],
            "src_idx_w": pc["src_idx_w"].reshape(16 * 128, LW),
            "dst_idx_w": pc["dst_idx_w"].reshape(16 * 128, LW),
            "ends_w": pc["ends_w"],
            "M": pc["M"],
            "runstart": rs.astype(ml_dtypes.bfloat16),
        })
    return maps


# ===================== runner =====================
_CACHE = {}

def _run_spmd(nc, maps):
    from concourse.bass_utils import run_bass_kernel_spmd
    return run_bass_kernel_spmd(nc, maps, list(range(8)))


def kernel(**inputs):
    x = np.asarray(inputs["x"], np.float32)
    N = x.shape[0]
    if np.any(np.asarray(inputs["b1"])) or N % 8:
        return _kernel_numpy(**inputs)
    pp = prep(np.asarray(inputs["edge_index"]), N)
    nc = build(pp, dbg=False)
    maps = make_in_maps(pp, inputs)
    res = _run_spmd(nc, maps)
    Nsh = pp["Nsh"]
    y = np.zeros((N, 1), np.float32)
    for c in range(8):
        y[c * Nsh:(c + 1) * Nsh, 0] = res.results[c]["y"].reshape(-1)[:Nsh]
    return y


def _kernel_numpy(x, edge_index, W1, a_src1, a_dst1, b1, W2, a_src2, a_dst2,
                  b2, Wl, bl):
    def lr(v):
        return np.where(v > 0, v, 0.2 * v).astype(np.float32)

    def conv(h, src, dst, W, asrc, adst, b, n):
        hh = (h @ W).astype(np.float32)
        u, v = hh @ asrc, hh @ adst
        e = lr(u[src] + v[dst])
        m = np.full(n, -np.inf, np.float32)
        np.maximum.at(m, dst, e)
        ee = np.exp(e - m[dst]).astype(np.float32)
        den = np.bincount(dst, weights=ee, minlength=n).astype(np.float32)
        al = ee / (den[dst] + 1e-16)
        out = np.zeros((n, hh.shape[1]), np.float32)
        wh = hh[src] * al[:, None]
        for k in range(hh.shape[1]):
            out[:, k] = np.bincount(dst, weights=wh[:, k], minlength=n)
        return out + b

    n = x.shape[0]
    loop = np.arange(n, dtype=np.int64)
    src = np.concatenate([edge_index[0], loop])
    dst = np.concatenate([edge_index[1], loop])
    h = np.maximum(conv(np.asarray(x, np.float32), src, dst, W1, a_src1,
                        a_dst1, b1, n), 0)
    h = np.maximum(conv(h, src, dst, W2, a_src2, a_dst2, b2, n), 0)
    return (h @ Wl + bl).astype(np.float32)

